# revision 11
# baseline (speedup 1.0000x reference)
"""Trainium2 Bass kernel for a TimeSformer-style divided space-time attention block.

Sharding: pure data-parallel over B (8 batch elements -> 8 NeuronCores), no
collectives. Each core computes the full block for one batch element.

Layout strategy per core:
  - residual stream kept token-major [tokens, 768] in DRAM (fp32)
  - matmul activations kept feature-major ("transposed", [feat, tokens]) bf16
    in SBUF; LayerNorm computes per-token stats token-major, then PE-transposes
  - attention uses the S^T trick: S^T = matmul(lhsT=K^T, rhs=Q^T) so softmax
    sums land on the matmul (ones column appended to V); no max-subtraction
    (post-LN logits are small), fp32 exp, bf16 matmul inputs everywhere
"""

import numpy as np
import ml_dtypes

import concourse.bass as bass
import concourse.mybir as mybir
import concourse.tile as tile
from concourse import bacc

F32 = mybir.dt.float32
BF16 = mybir.dt.bfloat16
AF = mybir.ActivationFunctionType
ALU = mybir.AluOpType
AX = mybir.AxisListType

D = 768
NH = 12
HD = 64
HID = 3072
B = 8
T = 8
HW = 196          # 14*14 spatial patches
N = 1569          # 1 + HW*T tokens in residual stream
NT = 1568         # temporal-attention tokens (HW*T)
NF = 197          # spatial frame length (1 cls + HW)
NS = T * NF       # 1576 spatial tokens
SCALE = HD ** -0.5
P = 128
EPS = 1e-5

# temporal batching: 16 sequences of length 8 -> 128-token groups
T_GROUPS = [(g * P, P) for g in range(12)] + [(12 * P, 32)]


def tiles_of(n, step=128):
    return [(i, min(step, n - i)) for i in range(0, n, step)]


def build_program(sim_gelu=False):
    nc = bacc.Bacc("TRN2", target_bir_lowering=False, debug=False, num_devices=8)

    def din(name, shape):
        return nc.dram_tensor(name, shape, F32, kind="ExternalInput").ap()

    x = din("x", [N, D])
    g1 = din("g1", [D]); b1 = din("b1", [D])
    Wqkv_s = din("Wqkv_s", [D, 3 * D]); Wproj_s = din("Wproj_s", [D, D]); bproj_s = din("bproj_s", [D])
    gt = din("gt", [D]); bt = din("bt", [D])
    Wqkv_t = din("Wqkv_t", [D, 3 * D]); Wproj_t = din("Wproj_t", [D, D]); bproj_t = din("bproj_t", [D])
    Wtfc = din("Wtfc", [D, D]); btfc = din("btfc", [D])
    g2 = din("g2", [D]); b2 = din("b2", [D])
    W1 = din("W1", [D, HID]); b1m = din("b1m", [HID])
    W2 = din("W2", [HID, D]); b2m = din("b2m", [D])
    maskbd = nc.dram_tensor("maskbd", [P, P], BF16, kind="ExternalInput").ap()
    ident_bf_d = nc.dram_tensor("ident_bf", [P, P], BF16, kind="ExternalInput").ap()
    ident_f_d = nc.dram_tensor("ident_f", [P, P], F32, kind="ExternalInput").ap()

    out = nc.dram_tensor("out", [N, D], F32, kind="ExternalOutput").ap()
    xt_d = nc.dram_tensor("xt_i", [NT, D], F32).ap()
    xs_d = nc.dram_tensor("xs_i", [NS, D], F32).ap()
    x2_d = nc.dram_tensor("x2_i", [N, D], F32).ap()

    with tile.TileContext(nc) as tc:
        with tc.tile_pool(name="const", bufs=1) as const:
            mask_sb = const.tile([P, P], BF16, tag="mask")
            nc.sync.dma_start(out=mask_sb[:], in_=maskbd)
            idb = const.tile([P, P], BF16, tag="idb")
            nc.sync.dma_start(out=idb[:], in_=ident_bf_d)
            idf = const.tile([P, P], F32, tag="idf")
            nc.sync.dma_start(out=idf[:], in_=ident_f_d)
            eps_sb = const.tile([P, 1], F32, tag="eps")
            nc.vector.memset(eps_sb[:], EPS)

            def load_vec(ap, L, tag):
                t = const.tile([P, L // P], F32, tag=tag)
                nc.sync.dma_start(out=t[:], in_=ap.rearrange("(a p) -> p a", p=P))
                return t

            gt_sb = load_vec(gt, D, "gt"); bt_sb = load_vec(bt, D, "bt")
            g1_sb = load_vec(g1, D, "g1"); b1_sb = load_vec(b1, D, "b1")
            g2_sb = load_vec(g2, D, "g2"); b2_sb = load_vec(b2, D, "b2")
            bprt_sb = load_vec(bproj_t, D, "bprt"); btfc_sb = load_vec(btfc, D, "btfc")
            bprs_sb = load_vec(bproj_s, D, "bprs"); b2m_sb = load_vec(b2m, D, "b2m")
            b1m_sb = load_vec(b1m, HID, "b1m")

            def load_wT(ap, K, M, tag, pool):
                """fp32 DRAM [K, M] -> K//128 bf16 SBUF tiles [128, M] (cast DMA)."""
                ts = []
                for i, (k0, pk) in enumerate(tiles_of(K)):
                    t = pool.tile([P, M], BF16, tag=f"{tag}{i}", name=f"{tag}{i}")
                    nc.gpsimd.dma_start(out=t[:], in_=ap[k0:k0 + pk, :])
                    ts.append(t)
                return ts

            def ln_to_lnT(pool, ps_tr, src_rows_fn, n_tok, g_sb, b_sb, lnT, name):
                """LayerNorm token tiles from DRAM -> feature-major bf16 lnT tiles."""
                for tok0, pt in tiles_of(n_tok):
                    xt_sb = pool.tile([P, D], F32, tag=f"{name}x", bufs=2)
                    nc.sync.dma_start(out=xt_sb[:pt], in_=src_rows_fn(tok0, pt))
                    s6 = pool.tile([P, 2, 6], F32, tag=f"{name}s6")
                    nc.vector.bn_stats(s6[:pt, 0], xt_sb[:pt, 0:384])
                    nc.vector.bn_stats(s6[:pt, 1], xt_sb[:pt, 384:768])
                    s2 = pool.tile([P, 2], F32, tag=f"{name}s2")
                    nc.vector.bn_aggr(s2[:pt], s6[:pt].rearrange("p a c -> p (a c)"))
                    std = pool.tile([P, 1], F32, tag=f"{name}sd")
                    nc.scalar.activation(std[:pt], s2[:pt, 1:2], AF.Sqrt, bias=eps_sb[:pt])
                    inv = pool.tile([P, 1], F32, tag=f"{name}iv")
                    nc.vector.reciprocal(inv[:pt], std[:pt])
                    xh = pool.tile([P, D], BF16, tag=f"{name}xh")
                    nc.vector.tensor_scalar(xh[:pt], xt_sb[:pt], s2[:pt, 0:1], inv[:pt],
                                            ALU.subtract, ALU.mult)
                    for j in range(6):
                        ps = ps_tr.tile([P, P], BF16, tag="tr")
                        nc.tensor.transpose(ps[:, :pt], xh[:pt, j * P:(j + 1) * P],
                                            idb[:pt, :pt])
                        nc.vector.tensor_scalar(lnT[j][:, tok0:tok0 + pt], ps[:, :pt],
                                                g_sb[:, j:j + 1], b_sb[:, j:j + 1],
                                                ALU.mult, ALU.add)

            def mm_wx(wT, rhsT, m_tiles, n_tok, ps_mm, evict, chunk=512):
                """psum[m, tok] = sum_k wT[k][:, m]^T rhsT[k][:, tok]."""
                for mi, (m0, pm) in enumerate(m_tiles):
                    for ch0, pc in tiles_of(n_tok, chunk):
                        ps = ps_mm.tile([P, chunk], F32, tag="mm")
                        for k in range(len(wT)):
                            nc.tensor.matmul(ps[:pm, :pc],
                                             wT[k][:, m0:m0 + pm],
                                             rhsT[k][:, ch0:ch0 + pc],
                                             start=(k == 0), stop=(k == len(wT) - 1))
                        evict(mi, m0, pm, ch0, pc, ps)

            # =========================================================
            # Stage T: temporal attention (196 sequences of length 8)
            # =========================================================
            with tc.tile_pool(name="t_main", bufs=1) as main, \
                 tc.tile_pool(name="t_work", bufs=3) as work, \
                 tc.tile_pool(name="t_mm", bufs=2, space="PSUM") as ps_mm, \
                 tc.tile_pool(name="t_tr", bufs=2, space="PSUM") as ps_tr, \
                 tc.tile_pool(name="t_st", bufs=2, space="PSUM") as ps_st, \
                 tc.tile_pool(name="t_ov", bufs=2, space="PSUM") as ps_ov:

                wqkvT = load_wT(Wqkv_t, D, 3 * D, "wqkvt", main)
                wprT = load_wT(Wproj_t, D, D, "wprt", main)
                wtfcT = load_wT(Wtfc, D, D, "wtfc", main)

                lnT = [main.tile([P, NT], BF16, tag=f"lnT{j}", name=f"lnT{j}") for j in range(6)]
                ln_to_lnT(work, ps_tr, lambda t0, pt: x[1 + t0:1 + t0 + pt, :], NT,
                          gt_sb, bt_sb, lnT, "lnt")

                qkT = [main.tile([P, NT], BF16, tag=f"qkT{j}", name=f"qkT{j}") for j in range(12)]

                def ev_qk(mi, m0, pm, ch0, pc, ps):
                    nc.scalar.activation(qkT[mi][:pm, ch0:ch0 + pc], ps[:pm, :pc], AF.Copy)
                mm_wx(wqkvT, lnT, tiles_of(2 * D), NT, ps_mm, ev_qk)

                v_t = [main.tile([P, NH, HD + 1], BF16, tag=f"vt{g}", name=f"vt{g}")
                       for g in range(len(T_GROUPS))]
                for g, (t0, pt) in enumerate(T_GROUPS):
                    nc.vector.memset(v_t[g][:pt, :, HD:HD + 1], 1.0)
                    for half in range(2):
                        ps = ps_mm.tile([P, 512], F32, tag="mm")
                        for k in range(6):
                            nc.tensor.matmul(
                                ps[:pt, :384],
                                lnT[k][:, t0:t0 + pt],
                                wqkvT[k][:, 2 * D + 384 * half:2 * D + 384 * (half + 1)],
                                start=(k == 0), stop=(k == 5))
                        nc.scalar.activation(
                            v_t[g][:pt, 6 * half:6 * (half + 1), 0:HD],
                            ps[:pt, :384].rearrange("p (a c) -> p a c", a=6), AF.Copy)

                oT = [main.tile([P, NT], BF16, tag=f"oT{j}", name=f"oT{j}") for j in range(6)]
                for g, (t0, pt) in enumerate(T_GROUPS):
                    o_tm = work.tile([P, D], BF16, tag="otm")
                    for h in range(NH):
                        j, r0 = h // 2, 64 * (h % 2)
                        st = ps_st.tile([P, P], F32, tag="st")
                        nc.tensor.matmul(st[:pt, :pt],
                                         qkT[6 + j][r0:r0 + HD, t0:t0 + pt],
                                         qkT[j][r0:r0 + HD, t0:t0 + pt],
                                         start=True, stop=True)
                        es = work.tile([P, P], BF16, tag="es")
                        nc.scalar.activation(es[:pt, :pt], st[:pt, :pt], AF.Exp, scale=SCALE)
                        nc.vector.tensor_tensor(es[:pt, :pt], es[:pt, :pt],
                                                mask_sb[:pt, :pt], ALU.mult)
                        ov = ps_ov.tile([P, HD + 1], F32, tag="ov")
                        nc.tensor.matmul(ov[:pt, :], es[:pt, :pt], v_t[g][:pt, h, :],
                                         start=True, stop=True)
                        rec = work.tile([P, 1], F32, tag="rec")
                        nc.vector.reciprocal(rec[:pt], ov[:pt, HD:HD + 1])
                        nc.vector.tensor_scalar_mul(o_tm[:pt, HD * h:HD * (h + 1)],
                                                    ov[:pt, 0:HD], rec[:pt])
                    for j in range(6):
                        ps = ps_tr.tile([P, P], BF16, tag="tr")
                        nc.tensor.transpose(ps[:, :pt], o_tm[:pt, j * P:(j + 1) * P],
                                            idb[:pt, :pt])
                        nc.vector.tensor_copy(oT[j][:, t0:t0 + pt], ps[:, :pt])

                pT = [main.tile([P, NT], BF16, tag=f"pT{j}", name=f"pT{j}") for j in range(6)]

                def ev_proj(mi, m0, pm, ch0, pc, ps):
                    nc.vector.tensor_scalar(pT[mi][:pm, ch0:ch0 + pc], ps[:pm, :pc],
                                            1.0, bprt_sb[:pm, mi:mi + 1], ALU.mult, ALU.add)
                mm_wx(wprT, oT, tiles_of(D), NT, ps_mm, ev_proj)

                rtT = [main.tile([P, NT], BF16, tag=f"lnT{j}", name=f"rtT{j}") for j in range(6)]

                def ev_tfc(mi, m0, pm, ch0, pc, ps):
                    nc.vector.tensor_scalar(rtT[mi][:pm, ch0:ch0 + pc], ps[:pm, :pc],
                                            1.0, btfc_sb[:pm, mi:mi + 1], ALU.mult, ALU.add)
                mm_wx(wtfcT, pT, tiles_of(D), NT, ps_mm, ev_tfc)

                # epilogue: xt = x[1:] + rt ; write xt and frame-permuted xs
                xs_re = xs_d.rearrange("(t n) c -> n t c", t=T)
                for g, (t0, pt) in enumerate(T_GROUPS):
                    xrow = work.tile([P, D], F32, tag="exr", bufs=2)
                    nc.sync.dma_start(out=xrow[:pt], in_=x[1 + t0:1 + t0 + pt, :])
                    xt_sb = work.tile([P, D], F32, tag="ext", bufs=2)
                    for j in range(6):
                        ps = ps_tr.tile([P, P], BF16, tag="tr")
                        nc.tensor.transpose(ps[:pt, :], rtT[j][:, t0:t0 + pt], idb[:, :])
                        nc.vector.tensor_tensor(xt_sb[:pt, j * P:(j + 1) * P], ps[:pt, :],
                                                xrow[:pt, j * P:(j + 1) * P], ALU.add)
                    nc.sync.dma_start(out=xt_d[t0:t0 + pt, :], in_=xt_sb[:pt])
                    hw0 = t0 // T
                    nc.sync.dma_start(out=xs_re[1 + hw0:1 + hw0 + pt // T, :, :],
                                      in_=xt_sb[:pt])
                for t in range(T):
                    nc.sync.dma_start(out=xs_d[t * NF:t * NF + 1, :], in_=x[1:2, :])

            # =========================================================
            # Stage S: spatial attention (8 frames of 197 tokens)
            # =========================================================
            with tc.tile_pool(name="s_main", bufs=1) as main, \
                 tc.tile_pool(name="s_work", bufs=3) as work, \
                 tc.tile_pool(name="s_mm", bufs=2, space="PSUM") as ps_mm, \
                 tc.tile_pool(name="s_tr", bufs=2, space="PSUM") as ps_tr, \
                 tc.tile_pool(name="s_st", bufs=2, space="PSUM") as ps_st, \
                 tc.tile_pool(name="s_ov", bufs=2, space="PSUM") as ps_ov:

                wqkvS = load_wT(Wqkv_s, D, 3 * D, "wqkvs", main)
                wprS = load_wT(Wproj_s, D, D, "wprs", main)

                lnT = [main.tile([P, NS], BF16, tag=f"lnS{j}", name=f"lnS{j}") for j in range(6)]
                ln_to_lnT(work, ps_tr, lambda t0, pt: xs_d[t0:t0 + pt, :], NS,
                          g1_sb, b1_sb, lnT, "lns")

                qkT = [main.tile([P, NS], BF16, tag=f"qkS{j}", name=f"qkS{j}") for j in range(12)]

                def ev_qks(mi, m0, pm, ch0, pc, ps):
                    nc.scalar.activation(qkT[mi][:pm, ch0:ch0 + pc], ps[:pm, :pc], AF.Copy)
                mm_wx(wqkvS, lnT, tiles_of(2 * D), NS, ps_mm, ev_qks)

                v_s = [[main.tile([P, NH, HD + 1], BF16, tag=f"vs{t}_{i}", name=f"vs{t}_{i}")
                        for i in range(2)] for t in range(T)]
                for t in range(T):
                    f0 = t * NF
                    for i, (k0, pk) in enumerate(tiles_of(NF)):
                        nc.vector.memset(v_s[t][i][:pk, :, HD:HD + 1], 1.0)
                        for half in range(2):
                            ps = ps_mm.tile([P, 512], F32, tag="mm")
                            for k in range(6):
                                nc.tensor.matmul(
                                    ps[:pk, :384],
                                    lnT[k][:, f0 + k0:f0 + k0 + pk],
                                    wqkvS[k][:, 2 * D + 384 * half:2 * D + 384 * (half + 1)],
                                    start=(k == 0), stop=(k == 5))
                            nc.scalar.activation(
                                v_s[t][i][:pk, 6 * half:6 * (half + 1), 0:HD],
                                ps[:pk, :384].rearrange("p (a c) -> p a c", a=6), AF.Copy)

                oT = [main.tile([P, NS], BF16, tag=f"oS{j}", name=f"oS{j}") for j in range(6)]
                for t in range(T):
                    f0 = t * NF
                    for h in range(NH):
                        j, r0 = h // 2, 64 * (h % 2)
                        qs = qkT[j][r0:r0 + HD, f0:f0 + NF]
                        es_list = []
                        for i, (k0, pk) in enumerate(tiles_of(NF)):
                            st = ps_st.tile([P, NF], F32, tag="st")
                            nc.tensor.matmul(st[:pk, :NF],
                                             qkT[6 + j][r0:r0 + HD, f0 + k0:f0 + k0 + pk],
                                             qs, start=True, stop=True)
                            es = work.tile([P, NF], BF16, tag="esS")
                            nc.scalar.activation(es[:pk, :NF], st[:pk, :NF], AF.Exp,
                                                 scale=SCALE)
                            es_list.append((es, k0, pk))
                        ov = ps_ov.tile([HD + 1, NF], F32, tag="ov")
                        for i, (es, k0, pk) in enumerate(es_list):
                            nc.tensor.matmul(ov[:, :NF], v_s[t][i][:pk, h, :],
                                             es[:pk, :NF],
                                             start=(i == 0), stop=(i == len(es_list) - 1))
                        rec = work.tile([1, NF], F32, tag="recS")
                        nc.vector.reciprocal(rec[:1, :], ov[HD:HD + 1, :])
                        bc = work.tile([HD, NF], F32, tag="bcS")
                        nc.gpsimd.partition_broadcast(bc[:, :], rec[:1, :])
                        nc.vector.tensor_tensor(oT[j][r0:r0 + HD, f0:f0 + NF],
                                                ov[0:HD, :NF], bc[:, :], ALU.mult)

                rsT = [main.tile([P, NS], BF16, tag=f"lnS{j}", name=f"rsT{j}") for j in range(6)]

                def ev_projs(mi, m0, pm, ch0, pc, ps):
                    nc.vector.tensor_scalar(rsT[mi][:pm, ch0:ch0 + pc], ps[:pm, :pc],
                                            1.0, bprs_sb[:pm, mi:mi + 1], ALU.mult, ALU.add)
                mm_wx(wprS, oT, tiles_of(D), NS, ps_mm, ev_projs)

                # cls_out = mean over frames of rs cls columns; x2[0] = x[1] + cls_out
                cls6 = work.tile([P, 6], F32, tag="cls6")
                for j in range(6):
                    nc.vector.tensor_reduce(
                        cls6[:, j:j + 1],
                        rsT[j].rearrange("p (t n) -> p n t", t=T)[:, 0:1, :],
                        AX.X, ALU.add)
                nc.vector.tensor_scalar_mul(cls6[:, :], cls6[:, :], 1.0 / T)
                psc = ps_mm.tile([P, P], F32, tag="mm", name="psc")
                nc.tensor.transpose(psc[:6, :], cls6[:, 0:6], idf[:, :])
                x1r = work.tile([6, P], F32, tag="x1r")
                nc.sync.dma_start(out=x1r[:, :],
                                  in_=x[1:2, :].rearrange("a (p c) -> (a p) c", c=P))
                cls_tm = work.tile([6, P], F32, tag="clstm")
                nc.vector.tensor_tensor(cls_tm[:, :], psc[:6, :], x1r[:, :], ALU.add)
                nc.sync.dma_start(out=x2_d[0:1, :].rearrange("a (p c) -> (a p) c", c=P),
                                  in_=cls_tm[:, :])

                # epilogue: x2[1+8*hw+t] = xt[8*hw+t] + rs[t, 1+hw]
                xt_re = xt_d.rearrange("(n t) c -> n t c", t=T)
                x2_re = x2_d[1:N, :].rearrange("(n t) c -> n t c", t=T)
                for t in range(T):
                    f0 = t * NF
                    for (q0, pq) in tiles_of(HW):
                        xrow = work.tile([P, D], F32, tag="sxr", bufs=2)
                        nc.sync.dma_start(out=xrow[:pq], in_=xt_re[q0:q0 + pq, t:t + 1, :])
                        x2_sb = work.tile([P, D], F32, tag="sx2", bufs=2)
                        for j in range(6):
                            ps = ps_tr.tile([P, P], BF16, tag="tr")
                            nc.tensor.transpose(ps[:pq, :],
                                                rsT[j][:, f0 + 1 + q0:f0 + 1 + q0 + pq],
                                                idb[:, :])
                            nc.vector.tensor_tensor(x2_sb[:pq, j * P:(j + 1) * P],
                                                    ps[:pq, :],
                                                    xrow[:pq, j * P:(j + 1) * P], ALU.add)
                        nc.sync.dma_start(out=x2_re[q0:q0 + pq, t:t + 1, :], in_=x2_sb[:pq])

            # =========================================================
            # Stage M: MLP with exact GELU
            # =========================================================
            with tc.tile_pool(name="m_main", bufs=1) as main, \
                 tc.tile_pool(name="m_work", bufs=3) as work, \
                 tc.tile_pool(name="m_g", bufs=2) as gpool, \
                 tc.tile_pool(name="m_mm", bufs=3, space="PSUM") as ps_mm, \
                 tc.tile_pool(name="m_tr", bufs=3, space="PSUM") as ps_tr:

                w1T = load_wT(W1, D, HID, "w1", main)
                w2T = load_wT(W2, HID, D, "w2", main)

                lnT = [main.tile([P, N], BF16, tag=f"lnM{j}", name=f"lnM{j}") for j in range(6)]
                ln_to_lnT(work, ps_tr, lambda t0, pt: x2_d[t0:t0 + pt, :], N,
                          g2_sb, b2_sb, lnT, "lnm")

                for c0, pc in tiles_of(N, 512):
                    g1T = [gpool.tile([P, 512], BF16, tag=f"g1T{m}", name=f"g1T{m}") for m in range(24)]
                    for m in range(24):
                        ps = ps_mm.tile([P, 512], F32, tag="mm")
                        for k in range(6):
                            nc.tensor.matmul(ps[:, :pc], w1T[k][:, m * P:(m + 1) * P],
                                             lnT[k][:, c0:c0 + pc],
                                             start=(k == 0), stop=(k == 5))
                        if sim_gelu:
                            # CoreSim has no Gelu LUT: x*sigmoid(1.702x) approx
                            hb = work.tile([P, 512], F32, tag="hb")
                            nc.vector.tensor_scalar(hb[:, :pc], ps[:, :pc], 1.0,
                                                    b1m_sb[:, m:m + 1], ALU.mult, ALU.add)
                            sg = work.tile([P, 512], F32, tag="sg")
                            nc.scalar.activation(sg[:, :pc], hb[:, :pc], AF.Sigmoid,
                                                 scale=1.702)
                            nc.vector.tensor_tensor(g1T[m][:, :pc], hb[:, :pc],
                                                    sg[:, :pc], ALU.mult)
                        else:
                            nc.scalar.activation(g1T[m][:, :pc], ps[:, :pc], AF.Gelu,
                                                 bias=b1m_sb[:, m:m + 1])
                    o2T = []
                    for mi in range(6):
                        ps = ps_mm.tile([P, 512], F32, tag="mm")
                        for k in range(24):
                            nc.tensor.matmul(ps[:, :pc], w2T[k][:, mi * P:(mi + 1) * P],
                                             g1T[k][:, :pc],
                                             start=(k == 0), stop=(k == 23))
                        o2 = gpool.tile([P, 512], BF16, tag=f"o2{mi}")
                        nc.vector.tensor_scalar(o2[:, :pc], ps[:, :pc], 1.0,
                                                b2m_sb[:, mi:mi + 1], ALU.mult, ALU.add)
                        o2T.append(o2)
                    for q0, pq in tiles_of(pc):
                        xrow = work.tile([P, D], F32, tag="mxr", bufs=2)
                        nc.sync.dma_start(out=xrow[:pq], in_=x2_d[c0 + q0:c0 + q0 + pq, :])
                        ot_sb = work.tile([P, D], F32, tag="mot", bufs=2)
                        for j in range(6):
                            ps = ps_tr.tile([P, P], BF16, tag="tr")
                            nc.tensor.transpose(ps[:pq, :], o2T[j][:, q0:q0 + pq],
                                                idb[:, :])
                            nc.vector.tensor_tensor(ot_sb[:pq, j * P:(j + 1) * P],
                                                    ps[:pq, :],
                                                    xrow[:pq, j * P:(j + 1) * P], ALU.add)
                        nc.sync.dma_start(out=out[c0 + q0:c0 + q0 + pq, :], in_=ot_sb[:pq])

    nc.compile()
    return nc


def _null():
    from contextlib import nullcontext
    return nullcontext()


_CACHED = {}


def _get_program():
    if "nc" not in _CACHED:
        _CACHED["nc"] = build_program()
    return _CACHED["nc"]


def _host_consts():
    mask = np.kron(np.eye(16, dtype=np.float32), np.ones((8, 8), np.float32))
    ident = np.eye(P, dtype=np.float32)
    return {
        "maskbd": mask.astype(ml_dtypes.bfloat16),
        "ident_bf": ident.astype(ml_dtypes.bfloat16),
        "ident_f": ident,
    }


WNAMES = ["g1", "b1", "Wqkv_s", "Wproj_s", "bproj_s", "gt", "bt", "Wqkv_t",
          "Wproj_t", "bproj_t", "Wtfc", "btfc", "g2", "b2", "W1", "b1m",
          "W2", "b2m"]


def make_in_maps(inputs):
    consts = _host_consts()
    x = np.asarray(inputs["x"], np.float32)
    base = {k: np.ascontiguousarray(np.asarray(inputs[k], np.float32)) for k in WNAMES}
    base.update(consts)
    return [dict(base, x=np.ascontiguousarray(x[i])) for i in range(8)]


def kernel(**inputs):
    nc = _get_program()
    in_maps = make_in_maps(inputs)
    core_ids = list(range(8))
    from concourse.bass_utils import run_bass_kernel_spmd
    res = run_bass_kernel_spmd(nc, in_maps, core_ids)
    return np.stack([res.results[i]["out"] for i in core_ids], axis=0)


if __name__ == "__main__":
    build_program()
    print("built ok")


# revision 23
# speedup vs baseline: 113.2103x; 113.2103x over previous
"""Trainium2 Bass kernel for a TimeSformer-style divided space-time attention block.

Sharding: pure data-parallel over B (8 batch elements -> 8 NeuronCores), no
collectives. Each core computes the full block for one batch element.

Layout strategy per core:
  - residual stream token-major [tokens, 768] fp32 in DRAM; matmul activations
    feature-major ("transposed") bf16 in SBUF; LayerNorm computes per-token
    stats token-major then PE-transposes into the feature-major ln^T tiles
  - all weight@activation GEMMs run token-chunk-outer so consumers of a chunk
    start while later chunks still compute
  - attention uses the S^T trick: S^T = matmul(lhsT=K^T, rhs=Q^T); softmax
    denominators ride a ones-column appended to V; no max-subtraction (post-LN
    logits are small); fp32 exp/stats, bf16 matmul inputs
"""

import numpy as np
import ml_dtypes

import concourse.bass as bass
import concourse.mybir as mybir
import concourse.tile as tile
from concourse import bacc

F32 = mybir.dt.float32
BF16 = mybir.dt.bfloat16
AF = mybir.ActivationFunctionType
ALU = mybir.AluOpType
AX = mybir.AxisListType

D = 768
NH = 12
HD = 64
HID = 3072
B = 8
T = 8
HW = 196
N = 1569
NT = 1568
NF = 197
NS = T * NF
SCALE = HD ** -0.5
P = 128
EPS = 1e-5

T_GROUPS = [(g * P, P) for g in range(12)] + [(12 * P, 32)]


def tiles_of(n, step=128):
    return [(i, min(step, n - i)) for i in range(0, n, step)]


def build_program(sim_gelu=False, loop_n=0):
    nc = bacc.Bacc("TRN2", target_bir_lowering=False, debug=False, num_devices=8)

    def din(name, shape):
        return nc.dram_tensor(name, shape, F32, kind="ExternalInput").ap()

    x = din("x", [N, D])
    g1 = din("g1", [D]); b1 = din("b1", [D])
    Wqkv_s = din("Wqkv_s", [D, 3 * D]); Wproj_s = din("Wproj_s", [D, D]); bproj_s = din("bproj_s", [D])
    gt = din("gt", [D]); bt = din("bt", [D])
    Wqkv_t = din("Wqkv_t", [D, 3 * D]); Wproj_t = din("Wproj_t", [D, D]); bproj_t = din("bproj_t", [D])
    Wtfc = din("Wtfc", [D, D]); btfc = din("btfc", [D])
    g2 = din("g2", [D]); b2 = din("b2", [D])
    W1 = din("W1", [D, HID]); b1m = din("b1m", [HID])
    W2 = din("W2", [HID, D]); b2m = din("b2m", [D])
    maskbd = nc.dram_tensor("maskbd", [P, P], BF16, kind="ExternalInput").ap()
    ident_bf_d = nc.dram_tensor("ident_bf", [P, P], BF16, kind="ExternalInput").ap()
    ident_f_d = nc.dram_tensor("ident_f", [P, P], F32, kind="ExternalInput").ap()

    out = nc.dram_tensor("out", [N, D], F32, kind="ExternalOutput").ap()
    xt_d = nc.dram_tensor("xt_i", [NT, D], F32).ap()
    x2_d = nc.dram_tensor("x2_i", [N, D], F32).ap()

    from contextlib import nullcontext

    with tile.TileContext(nc) as tc:
      with tc.tile_pool(name="const", bufs=1) as const:
        # loads needed by the very first LN chain go first (head of HWDGE queue)
        idb = const.tile([P, P], BF16, tag="idb")
        nc.sync.dma_start(out=idb[:], in_=ident_bf_d)
        eps_sb = const.tile([P, 1], F32, tag="eps")
        nc.vector.memset(eps_sb[:], EPS)

        def load_vec(ap, L, tag):
            t = const.tile([P, L // P], F32, tag=tag, name=tag)
            nc.sync.dma_start(out=t[:], in_=ap.rearrange("(a p) -> p a", p=P))
            return t

        gt_sb = load_vec(gt, D, "gt"); bt_sb = load_vec(bt, D, "bt")
        consts = {}

        def load_late_consts():
            consts["mask"] = const.tile([P, P], BF16, tag="mask", name="mask_sb")
            nc.sync.dma_start(out=consts["mask"][:], in_=maskbd)
            consts["idf"] = const.tile([P, P], F32, tag="idf", name="idf")
            nc.sync.dma_start(out=consts["idf"][:], in_=ident_f_d)
            for nm, ap, L in [("g1", g1, D), ("b1", b1, D), ("g2", g2, D),
                              ("b2", b2, D), ("bprt", bproj_t, D), ("btfc", btfc, D),
                              ("bprs", bproj_s, D), ("b2m", b2m, D), ("b1m", b1m, HID)]:
                consts[nm] = load_vec(ap, L, nm)

        def load_wT(ap, K, M, tag, pool):
            ts = []
            for i, (k0, pk) in enumerate(tiles_of(K)):
                t = pool.tile([P, M], BF16, tag=f"{tag}{i}", name=f"{tag}{i}")
                nc.gpsimd.dma_start(out=t[:], in_=ap[k0:k0 + pk, :])
                ts.append(t)
            return ts

        def ln_to_lnT(pool, ps_tr, src_rows_fn, n_tok, g_sb, b_sb, lnT, name,
                      col0=0, cls_src=None):
            """LayerNorm token tiles from DRAM -> feature-major bf16 lnT tiles."""
            for tok0, pt in tiles_of(n_tok):
                x_sb = pool.tile([P, D], F32, tag=f"{name}x", name=f"{name}x", bufs=2)
                if cls_src is not None and tok0 == 0:
                    nc.sync.dma_start(out=x_sb[0:1], in_=cls_src)
                    nc.sync.dma_start(out=x_sb[1:pt], in_=src_rows_fn(1, pt - 1))
                else:
                    nc.sync.dma_start(out=x_sb[:pt], in_=src_rows_fn(tok0, pt))
                s6 = pool.tile([P, 2, 6], F32, tag=f"{name}s6", name=f"{name}s6")
                nc.vector.bn_stats(s6[:pt, 0], x_sb[:pt, 0:384])
                nc.vector.bn_stats(s6[:pt, 1], x_sb[:pt, 384:768])
                s2 = pool.tile([P, 2], F32, tag=f"{name}s2", name=f"{name}s2")
                nc.vector.bn_aggr(s2[:pt], s6[:pt].rearrange("p a c -> p (a c)"))
                std = pool.tile([P, 1], F32, tag=f"{name}sd", name=f"{name}sd")
                nc.scalar.activation(std[:pt], s2[:pt, 1:2], AF.Sqrt, bias=eps_sb[:pt])
                inv = pool.tile([P, 1], F32, tag=f"{name}iv", name=f"{name}iv")
                nc.vector.reciprocal(inv[:pt], std[:pt])
                xh = pool.tile([P, D], BF16, tag=f"{name}xh", name=f"{name}xh", bufs=2)
                nc.vector.tensor_scalar(xh[:pt], x_sb[:pt], s2[:pt, 0:1], inv[:pt],
                                        ALU.subtract, ALU.mult)
                for j in range(6):
                    ps = ps_tr.tile([P, P], BF16, tag="tr", name="trp")
                    nc.tensor.transpose(ps[:, :pt], xh[:pt, j * P:(j + 1) * P],
                                        idb[:pt, :pt])
                    nc.vector.tensor_scalar(lnT[j][:, col0 + tok0:col0 + tok0 + pt],
                                            ps[:, :pt],
                                            g_sb[:, j:j + 1], b_sb[:, j:j + 1],
                                            ALU.mult, ALU.add)

        def mm_wx(wT, rhsT, m_tiles, n_tok, ps_mm, evict, chunk=512):
            """psum[m, tok] = sum_k wT[k][:, m]^T rhs[k][:, tok]; chunk-outer."""
            for ch0, pc in tiles_of(n_tok, chunk):
                for mi, (m0, pm) in enumerate(m_tiles):
                    ps = ps_mm.tile([P, chunk], F32, tag="mm", name="mmps")
                    for k in range(len(wT)):
                        nc.tensor.matmul(ps[:pm, :pc],
                                         wT[k][:, m0:m0 + pm],
                                         rhsT[k][:, ch0:ch0 + pc],
                                         start=(k == 0), stop=(k == len(wT) - 1))
                    evict(mi, m0, pm, ch0, pc, ps)

        loop_cm = tc.For_i(0, loop_n, 1) if loop_n else nullcontext()
        with loop_cm:
            # =====================================================
            # Stage T: temporal attention (196 sequences of len 8)
            # =====================================================
            with tc.tile_pool(name="t_main", bufs=1) as main, \
                 tc.tile_pool(name="t_work", bufs=3) as work, \
                 tc.tile_pool(name="t_mm", bufs=2, space="PSUM") as ps_mm, \
                 tc.tile_pool(name="t_tr", bufs=2, space="PSUM") as ps_tr, \
                 tc.tile_pool(name="t_st", bufs=2, space="PSUM") as ps_st, \
                 tc.tile_pool(name="t_ov", bufs=2, space="PSUM") as ps_ov:

                lnT = [main.tile([P, NT], BF16, tag=f"lnT{j}", name=f"lnT{j}")
                       for j in range(6)]
                ln_to_lnT(work, ps_tr, lambda t0, pt: x[1 + t0:1 + t0 + pt, :], NT,
                          gt_sb, bt_sb, lnT, "lnt")

                load_late_consts()
                wqkvT = load_wT(Wqkv_t, D, 3 * D, "wqkvt", main)
                wprT = load_wT(Wproj_t, D, D, "wprt", main)
                wtfcT = load_wT(Wtfc, D, D, "wtfc", main)

                qkT = [main.tile([P, NT], BF16, tag=f"qkT{j}", name=f"qkT{j}")
                       for j in range(12)]

                def ev_qk(mi, m0, pm, ch0, pc, ps):
                    nc.scalar.activation(qkT[mi][:pm, ch0:ch0 + pc], ps[:pm, :pc],
                                         AF.Copy)
                mm_wx(wqkvT, lnT, tiles_of(2 * D), NT, ps_mm, ev_qk)

                v_t = [main.tile([P, NH, HD + 1], BF16, tag=f"vt{g}", name=f"vt{g}")
                       for g in range(len(T_GROUPS))]
                for g, (t0, pt) in enumerate(T_GROUPS):
                    nc.vector.memset(v_t[g][:pt, :, HD:HD + 1], 1.0)
                    for half in range(2):
                        ps = ps_mm.tile([P, 512], F32, tag="mm", name="vtps")
                        for k in range(6):
                            nc.tensor.matmul(
                                ps[:pt, :384],
                                lnT[k][:, t0:t0 + pt],
                                wqkvT[k][:, 2 * D + 384 * half:2 * D + 384 * (half + 1)],
                                start=(k == 0), stop=(k == 5))
                        nc.scalar.activation(
                            v_t[g][:pt, 6 * half:6 * (half + 1), 0:HD],
                            ps[:pt, :384].rearrange("p (a c) -> p a c", a=6), AF.Copy)

                oT = [main.tile([P, NT], BF16, tag=f"oT{j}", name=f"oT{j}")
                      for j in range(6)]
                for g, (t0, pt) in enumerate(T_GROUPS):
                    o_tm = work.tile([P, D], BF16, tag="otm", name="otm", bufs=2)
                    for h in range(NH):
                        j, r0 = h // 2, 64 * (h % 2)
                        st = ps_st.tile([P, P], F32, tag="st", name="stps")
                        nc.tensor.matmul(st[:pt, :pt],
                                         qkT[6 + j][r0:r0 + HD, t0:t0 + pt],
                                         qkT[j][r0:r0 + HD, t0:t0 + pt],
                                         start=True, stop=True)
                        es = work.tile([P, P], BF16, tag="es", name="es")
                        nc.scalar.activation(es[:pt, :pt], st[:pt, :pt], AF.Exp,
                                             scale=SCALE)
                        nc.gpsimd.tensor_tensor(es[:pt, :pt], es[:pt, :pt],
                                                consts["mask"][:pt, :pt], ALU.mult)
                        ov = ps_ov.tile([P, HD + 1], F32, tag="ov", name="ovps")
                        nc.tensor.matmul(ov[:pt, :], es[:pt, :pt], v_t[g][:pt, h, :],
                                         start=True, stop=True)
                        rec = work.tile([P, 1], F32, tag="rec", name="rec")
                        nc.vector.reciprocal(rec[:pt], ov[:pt, HD:HD + 1])
                        nc.vector.tensor_scalar_mul(o_tm[:pt, HD * h:HD * (h + 1)],
                                                    ov[:pt, 0:HD], rec[:pt])
                    for j in range(6):
                        ps = ps_tr.tile([P, P], BF16, tag="tr", name="otr")
                        nc.tensor.transpose(ps[:, :pt], o_tm[:pt, j * P:(j + 1) * P],
                                            idb[:pt, :pt])
                        nc.vector.tensor_copy(oT[j][:, t0:t0 + pt], ps[:, :pt])

                pT = [main.tile([P, NT], BF16, tag=f"pT{j}", name=f"pT{j}")
                      for j in range(6)]

                def ev_proj(mi, m0, pm, ch0, pc, ps):
                    nc.vector.tensor_scalar(pT[mi][:pm, ch0:ch0 + pc], ps[:pm, :pc],
                                            1.0, consts["bprt"][:pm, mi:mi + 1],
                                            ALU.mult, ALU.add)
                mm_wx(wprT, oT, tiles_of(D), NT, ps_mm, ev_proj)

                rtT = [main.tile([P, NT], BF16, tag=f"lnT{j}", name=f"rtT{j}")
                       for j in range(6)]

                def ev_tfc(mi, m0, pm, ch0, pc, ps):
                    nc.vector.tensor_scalar(rtT[mi][:pm, ch0:ch0 + pc], ps[:pm, :pc],
                                            1.0, consts["btfc"][:pm, mi:mi + 1],
                                            ALU.mult, ALU.add)
                mm_wx(wtfcT, pT, tiles_of(D), NT, ps_mm, ev_tfc)

                # epilogue: xt = x[1:] + rt -> xt_d (token-major)
                for g, (t0, pt) in enumerate(T_GROUPS):
                    xrow = work.tile([P, D], F32, tag="exr", name="exr", bufs=2)
                    nc.sync.dma_start(out=xrow[:pt], in_=x[1 + t0:1 + t0 + pt, :])
                    xt_sb = work.tile([P, D], F32, tag="ext", name="ext", bufs=2)
                    for j in range(6):
                        ps = ps_tr.tile([P, P], BF16, tag="tr", name="etr")
                        nc.tensor.transpose(ps[:pt, :], rtT[j][:, t0:t0 + pt], idb[:, :])
                        nc.vector.tensor_tensor(xt_sb[:pt, j * P:(j + 1) * P],
                                                ps[:pt, :],
                                                xrow[:pt, j * P:(j + 1) * P], ALU.add)
                    nc.sync.dma_start(out=xt_d[t0:t0 + pt, :], in_=xt_sb[:pt])

            # =====================================================
            # Stage S: spatial attention (8 frames of 197 tokens)
            # =====================================================
            with tc.tile_pool(name="s_main", bufs=1) as main, \
                 tc.tile_pool(name="s_work", bufs=3) as work, \
                 tc.tile_pool(name="s_mm", bufs=2, space="PSUM") as ps_mm, \
                 tc.tile_pool(name="s_tr", bufs=2, space="PSUM") as ps_tr, \
                 tc.tile_pool(name="s_st", bufs=2, space="PSUM") as ps_st, \
                 tc.tile_pool(name="s_ov", bufs=2, space="PSUM") as ps_ov:

                # LN reads xt directly through the frame-permuted AP (no xs
                # round trip); frame token 0 is the cls row x[1]
                lnS = [main.tile([P, NS], BF16, tag=f"lnS{j}", name=f"lnS{j}")
                       for j in range(6)]
                xt_ref = xt_d.rearrange("(n t) c -> n t c", t=T)
                for fr in range(T):
                    f0 = fr * NF
                    for (tk0, ptk) in tiles_of(NF):
                        def src_s(tok0, pt, fr=fr, tk0=tk0):
                            j0 = tk0 + tok0
                            return xt_ref[j0 - 1:j0 - 1 + pt, fr:fr + 1, :]
                        ln_to_lnT(work, ps_tr, src_s, ptk, consts["g1"], consts["b1"],
                                  lnS, "lns", col0=f0 + tk0,
                                  cls_src=(x[1:2, :] if tk0 == 0 else None))

                wqkvS = load_wT(Wqkv_s, D, 3 * D, "wqkvs", main)
                wprS = load_wT(Wproj_s, D, D, "wprs", main)

                qkT = [main.tile([P, NS], BF16, tag=f"qkS{j}", name=f"qkS{j}")
                       for j in range(12)]

                def ev_qks(mi, m0, pm, ch0, pc, ps):
                    nc.scalar.activation(qkT[mi][:pm, ch0:ch0 + pc], ps[:pm, :pc],
                                         AF.Copy)
                mm_wx(wqkvS, lnS, tiles_of(2 * D), NS, ps_mm, ev_qks)

                v_s = [[main.tile([P, NH, HD + 1], BF16, tag=f"vs{t}_{i}",
                                  name=f"vs{t}_{i}") for i in range(2)]
                       for t in range(T)]
                for t in range(T):
                    f0 = t * NF
                    for i, (k0, pk) in enumerate(tiles_of(NF)):
                        nc.vector.memset(v_s[t][i][:pk, :, HD:HD + 1], 1.0)
                        for half in range(2):
                            ps = ps_mm.tile([P, 512], F32, tag="mm", name="vsps")
                            for k in range(6):
                                nc.tensor.matmul(
                                    ps[:pk, :384],
                                    lnS[k][:, f0 + k0:f0 + k0 + pk],
                                    wqkvS[k][:, 2 * D + 384 * half:2 * D + 384 * (half + 1)],
                                    start=(k == 0), stop=(k == 5))
                            nc.scalar.activation(
                                v_s[t][i][:pk, 6 * half:6 * (half + 1), 0:HD],
                                ps[:pk, :384].rearrange("p (a c) -> p a c", a=6),
                                AF.Copy)

                oT = [main.tile([P, NS], BF16, tag=f"oS{j}", name=f"oS{j}")
                      for j in range(6)]
                for t in range(T):
                    f0 = t * NF
                    for h in range(NH):
                        j, r0 = h // 2, 64 * (h % 2)
                        qs = qkT[j][r0:r0 + HD, f0:f0 + NF]
                        es_list = []
                        for i, (k0, pk) in enumerate(tiles_of(NF)):
                            st = ps_st.tile([P, NF], F32, tag="st", name="stS")
                            nc.tensor.matmul(st[:pk, :NF],
                                             qkT[6 + j][r0:r0 + HD,
                                                        f0 + k0:f0 + k0 + pk],
                                             qs, start=True, stop=True)
                            es = work.tile([P, NF], BF16, tag="esS", name="esS")
                            nc.scalar.activation(es[:pk, :NF], st[:pk, :NF], AF.Exp,
                                                 scale=SCALE)
                            es_list.append((es, k0, pk))
                        ov = ps_ov.tile([HD + 1, NF], F32, tag="ov", name="ovS")
                        for i, (es, k0, pk) in enumerate(es_list):
                            nc.tensor.matmul(ov[:, :NF], v_s[t][i][:pk, h, :],
                                             es[:pk, :NF],
                                             start=(i == 0), stop=(i == len(es_list) - 1))
                        rec = work.tile([1, NF], F32, tag="recS", name="recS")
                        nc.vector.reciprocal(rec[:1, :], ov[HD:HD + 1, :])
                        bc = work.tile([HD, NF], F32, tag="bcS", name="bcS")
                        nc.gpsimd.partition_broadcast(bc[:, :], rec[:1, :])
                        nc.vector.tensor_tensor(oT[j][r0:r0 + HD, f0:f0 + NF],
                                                ov[0:HD, :NF], bc[:, :], ALU.mult)

                rsT = [main.tile([P, NS], BF16, tag=f"lnS{j}", name=f"rsT{j}")
                       for j in range(6)]

                def ev_projs(mi, m0, pm, ch0, pc, ps):
                    nc.vector.tensor_scalar(rsT[mi][:pm, ch0:ch0 + pc], ps[:pm, :pc],
                                            1.0, consts["bprs"][:pm, mi:mi + 1],
                                            ALU.mult, ALU.add)
                mm_wx(wprS, oT, tiles_of(D), NS, ps_mm, ev_projs)

                # cls_out = mean over frames of rs cls cols; x2[0] = x[1] + cls_out
                cls6 = work.tile([P, 6], F32, tag="cls6", name="cls6")
                for j in range(6):
                    nc.vector.tensor_reduce(
                        cls6[:, j:j + 1],
                        rsT[j].rearrange("p (t n) -> p n t", t=T)[:, 0:1, :],
                        AX.X, ALU.add)
                nc.vector.tensor_scalar_mul(cls6[:, :], cls6[:, :], 1.0 / T)
                psc = ps_mm.tile([P, P], F32, tag="mm", name="psc")
                nc.tensor.transpose(psc[:6, :], cls6[:, 0:6], consts["idf"][:, :])
                x1r = work.tile([6, P], F32, tag="x1r", name="x1r")
                nc.sync.dma_start(out=x1r[:, :],
                                  in_=x[1:2, :].rearrange("a (p c) -> (a p) c", c=P))
                cls_tm = work.tile([6, P], F32, tag="clstm", name="clstm")
                nc.vector.tensor_tensor(cls_tm[:, :], psc[:6, :], x1r[:, :], ALU.add)
                nc.sync.dma_start(out=x2_d[0:1, :].rearrange("a (p c) -> (a p) c", c=P),
                                  in_=cls_tm[:, :])

                # epilogue: x2[1+8*hw+t] = xt[8*hw+t] + rs[t, 1+hw]
                xt_re = xt_d.rearrange("(n t) c -> n t c", t=T)
                x2_re = x2_d[1:N, :].rearrange("(n t) c -> n t c", t=T)
                for (q0, pq) in tiles_of(HW):
                    for t in range(T):
                        f0 = t * NF
                        xrow = work.tile([P, D], F32, tag="sxr", name="sxr", bufs=2)
                        nc.sync.dma_start(out=xrow[:pq], in_=xt_re[q0:q0 + pq, t:t + 1, :])
                        x2_sb = work.tile([P, D], F32, tag="sx2", name="sx2", bufs=2)
                        for j in range(6):
                            ps = ps_tr.tile([P, P], BF16, tag="tr", name="str")
                            nc.tensor.transpose(ps[:pq, :],
                                                rsT[j][:, f0 + 1 + q0:f0 + 1 + q0 + pq],
                                                idb[:, :])
                            nc.vector.tensor_tensor(x2_sb[:pq, j * P:(j + 1) * P],
                                                    ps[:pq, :],
                                                    xrow[:pq, j * P:(j + 1) * P],
                                                    ALU.add)
                        nc.sync.dma_start(out=x2_re[q0:q0 + pq, t:t + 1, :],
                                          in_=x2_sb[:pq])

            # =====================================================
            # Stage M: MLP with exact GELU
            # =====================================================
            with tc.tile_pool(name="m_main", bufs=1) as main, \
                 tc.tile_pool(name="m_work", bufs=3) as work, \
                 tc.tile_pool(name="m_g", bufs=2) as gpool, \
                 tc.tile_pool(name="m_mm", bufs=3, space="PSUM") as ps_mm, \
                 tc.tile_pool(name="m_tr", bufs=3, space="PSUM") as ps_tr:

                lnM = [main.tile([P, N], BF16, tag=f"lnM{j}", name=f"lnM{j}")
                       for j in range(6)]
                ln_to_lnT(work, ps_tr, lambda t0, pt: x2_d[t0:t0 + pt, :], N,
                          consts["g2"], consts["b2"], lnM, "lnm")

                w1T = load_wT(W1, D, HID, "w1", main)
                w2T = load_wT(W2, HID, D, "w2", main)

                for c0, pc in tiles_of(N, 512):
                    g1T = [gpool.tile([P, 512], BF16, tag=f"g1T{m}", name=f"g1T{m}")
                           for m in range(24)]
                    for m in range(24):
                        ps = ps_mm.tile([P, 512], F32, tag="mm", name="f1ps")
                        for k in range(6):
                            nc.tensor.matmul(ps[:, :pc], w1T[k][:, m * P:(m + 1) * P],
                                             lnM[k][:, c0:c0 + pc],
                                             start=(k == 0), stop=(k == 5))
                        if sim_gelu:
                            hb = work.tile([P, 512], F32, tag="hb", name="hb")
                            nc.vector.tensor_scalar(hb[:, :pc], ps[:, :pc], 1.0,
                                                    consts["b1m"][:, m:m + 1],
                                                    ALU.mult, ALU.add)
                            sg = work.tile([P, 512], F32, tag="sg", name="sg")
                            nc.scalar.activation(sg[:, :pc], hb[:, :pc], AF.Sigmoid,
                                                 scale=1.702)
                            nc.vector.tensor_tensor(g1T[m][:, :pc], hb[:, :pc],
                                                    sg[:, :pc], ALU.mult)
                        else:
                            nc.scalar.activation(g1T[m][:, :pc], ps[:, :pc], AF.Gelu,
                                                 bias=consts["b1m"][:, m:m + 1])
                    o2T = []
                    for mi in range(6):
                        ps = ps_mm.tile([P, 512], F32, tag="mm", name="f2ps")
                        for k in range(24):
                            nc.tensor.matmul(ps[:, :pc], w2T[k][:, mi * P:(mi + 1) * P],
                                             g1T[k][:, :pc],
                                             start=(k == 0), stop=(k == 23))
                        o2 = gpool.tile([P, 512], BF16, tag=f"o2{mi}", name=f"o2{mi}")
                        nc.vector.tensor_scalar(o2[:, :pc], ps[:, :pc], 1.0,
                                                consts["b2m"][:, mi:mi + 1],
                                                ALU.mult, ALU.add)
                        o2T.append(o2)
                    for q0, pq in tiles_of(pc):
                        xrow = work.tile([P, D], F32, tag="mxr", name="mxr", bufs=2)
                        nc.sync.dma_start(out=xrow[:pq],
                                          in_=x2_d[c0 + q0:c0 + q0 + pq, :])
                        ot_sb = work.tile([P, D], F32, tag="mot", name="mot", bufs=2)
                        for j in range(6):
                            ps = ps_tr.tile([P, P], BF16, tag="tr", name="mtr")
                            nc.tensor.transpose(ps[:pq, :], o2T[j][:, q0:q0 + pq],
                                                idb[:, :])
                            nc.vector.tensor_tensor(ot_sb[:pq, j * P:(j + 1) * P],
                                                    ps[:pq, :],
                                                    xrow[:pq, j * P:(j + 1) * P],
                                                    ALU.add)
                        nc.sync.dma_start(out=out[c0 + q0:c0 + q0 + pq, :],
                                          in_=ot_sb[:pq])

    nc.compile()
    return nc


_CACHED = {}


def _get_program():
    if "nc" not in _CACHED:
        _CACHED["nc"] = build_program()
    return _CACHED["nc"]


def _host_consts():
    mask = np.kron(np.eye(16, dtype=np.float32), np.ones((8, 8), np.float32))
    ident = np.eye(P, dtype=np.float32)
    return {
        "maskbd": mask.astype(ml_dtypes.bfloat16),
        "ident_bf": ident.astype(ml_dtypes.bfloat16),
        "ident_f": ident,
    }


WNAMES = ["g1", "b1", "Wqkv_s", "Wproj_s", "bproj_s", "gt", "bt", "Wqkv_t",
          "Wproj_t", "bproj_t", "Wtfc", "btfc", "g2", "b2", "W1", "b1m",
          "W2", "b2m"]


def make_in_maps(inputs):
    consts = _host_consts()
    x = np.asarray(inputs["x"], np.float32)
    base = {k: np.ascontiguousarray(np.asarray(inputs[k], np.float32)) for k in WNAMES}
    base.update(consts)
    return [dict(base, x=np.ascontiguousarray(x[i])) for i in range(8)]


def kernel(**inputs):
    nc = _get_program()
    in_maps = make_in_maps(inputs)
    core_ids = list(range(8))
    from concourse.bass_utils import run_bass_kernel_spmd
    res = run_bass_kernel_spmd(nc, in_maps, core_ids)
    return np.stack([res.results[i]["out"] for i in core_ids], axis=0)


if __name__ == "__main__":
    build_program()
    print("built ok")
